# revision 3
# baseline (speedup 1.0000x reference)
"""Distributed triangular matmul C = tril(tril(A) @ tril(B)), N=4096, on 8 TRN2 cores.

Decomposition: output units (m, c2) = 128-row strip x 1024-col tile, for the
block-lower-triangle (m >= 8*c2). Each unit's k-range [8*c2, m] is split into
aligned 8-k-tile windows (1024 k each) -> "pieces". All 160 pieces are
identical in shape (8 k-steps x 2 psum banks), so an identical SPMD program
runs on all 8 cores (20 pieces each, grouped 8+8+4 under 3 resident B
windows); per-core semantics live entirely in host-packed input data.
Out-of-triangle k-steps multiply zero blocks of tril A / tril B - harmless.
Host sums the piece partials per unit (<= 4 pieces/unit).

Compute dtype: float32r (TF32-like, 1-8-11 in top 20 bits) - full PE rate at
free dim 512. DVE tensor_copy fp32->fp32r performs the rounding the BIR
verifier requires, and collapses matmul sync-waits onto one DVE semaphore
(the fp32r matmul's internal LDW has a tiny sync-wait slot budget).
"""

import sys

sys.path.insert(0, "/opt/trn_rl_repo")

import numpy as np

N = 4096
P = 128                  # partition / k-tile / m-strip size
CW = 1024                # unit column width (2 psum banks)
NM = N // P              # 32 m-strips
NC2 = N // CW            # 4 column tiles
WK = 8                   # k-tiles per window
SLOTS = (8, 8, 4)        # pieces per window slot (template, all cores)
NPIECES = sum(SLOTS)     # 20 per core
NCORES = 8


def _schedule():
    """Global list of (c2, j, m) pieces chunked into the uniform template.

    Returns per-core: windows  [3 x (c2, j)], pieces [20 x (c2, j, m)].
    """
    chunks8 = []
    chunks4 = []
    for c2 in range(NC2):
        for j in range(c2, 4):
            ms = list(range(8 * j, NM))
            if j < 3:
                for s in range(0, len(ms), 8):
                    chunks8.append(((c2, j), ms[s : s + 8]))
            else:
                for s in range(0, len(ms), 4):
                    chunks4.append(((c2, j), ms[s : s + 4]))
    assert len(chunks8) == 16 and len(chunks4) == 8
    cores = []
    for i in range(NCORES):
        sel = [chunks8[2 * i], chunks8[2 * i + 1], chunks4[i]]
        windows = [w for w, _ in sel]
        pieces = [(w[0], w[1], m) for w, ms in sel for m in ms]
        assert len(pieces) == NPIECES
        cores.append((windows, pieces))
    return cores


_CORES = _schedule()


def _build_program():
    import concourse.mybir as mybir
    import concourse.tile as tile
    from concourse import bacc

    f32 = mybir.dt.float32
    f32r = mybir.dt.float32r

    nc = bacc.Bacc(None, target_bir_lowering=False)
    a_in = nc.declare_dram_parameter("a", [NPIECES, P, WK * P], f32, isOutput=False)
    b_in = nc.declare_dram_parameter("b", [len(SLOTS), P, WK * CW], f32, isOutput=False)
    o_out = nc.declare_dram_parameter("o", [NPIECES, P, CW], f32, isOutput=True)

    starts = [sum(SLOTS[:i]) for i in range(len(SLOTS))]
    with tile.TileContext(nc) as tc:
        with (
            tc.tile_pool(name="braw", bufs=1) as braw_pool,
            tc.tile_pool(name="bwin", bufs=2) as bwin_pool,
            tc.tile_pool(name="araw", bufs=4) as araw_pool,
            tc.tile_pool(name="ar", bufs=3) as ar_pool,
            tc.tile_pool(name="out", bufs=3) as out_pool,
            tc.tile_pool(name="ps", bufs=4, space="PSUM") as ps_pool,
        ):
            braws, bwins, araws, ats = {}, {}, {}, {}

            # Software-pipelined emission: fp32r rounding copies (DVE) for
            # piece p+1 are queued ahead of piece p's psum evacuation
            # (ScalarE), so the DVE FIFO never stalls the PE chain. Output
            # DMAs ride the gpsimd SWDGE; A/B loads the sync HWDGE.
            def b_dma(s):
                braws[s] = braw_pool.tile([P, WK, CW], f32, tag="braw", name=f"braw{s}")
                nc.sync.dma_start(
                    braws[s][:], b_in[s].rearrange("p (w c) -> p w c", w=WK)
                )
                bwins[s] = bwin_pool.tile(
                    [P, WK, CW], f32r, tag="bwin", name=f"bwin{s}"
                )

            def b_round(s, c):
                nc.vector.tensor_copy(bwins[s][:, c, :], braws[s][:, c, :])

            def a_dma(p):
                araws[p] = araw_pool.tile([P, WK, P], f32, tag="araw", name=f"araw{p}")
                nc.sync.dma_start(
                    araws[p][:], a_in[p].rearrange("p (w m) -> p w m", w=WK)
                )

            def a_round(p):
                ats[p] = ar_pool.tile([P, WK, P], f32r, tag="ar", name=f"ar{p}")
                nc.vector.tensor_copy(ats[p][:], araws[p][:])

            b_dma(0)
            for c in range(WK):
                b_round(0, c)
            a_dma(0)
            a_round(0)
            a_dma(1)

            for s, npc in enumerate(SLOTS):
                for q in range(npc):
                    p = starts[s] + q
                    if q == 0 and s + 1 < len(SLOTS):
                        b_dma(s + 1)
                    if s + 1 < len(SLOTS):
                        cpp = (WK + npc - 1) // npc
                        for c in range(cpp * q, min(cpp * (q + 1), WK)):
                            b_round(s + 1, c)
                    if p + 2 < NPIECES:
                        a_dma(p + 2)
                    if p + 1 < NPIECES:
                        a_round(p + 1)
                    pst = ps_pool.tile([P, CW], f32, tag="ps")
                    for t in range(WK):
                        for h in range(2):
                            nc.tensor.matmul(
                                pst[:, 512 * h : 512 * (h + 1)],
                                ats[p][:, t, :],
                                bwins[s][:, t, 512 * h : 512 * (h + 1)],
                                start=(t == 0),
                                stop=(t == WK - 1),
                            )
                    ot = out_pool.tile([P, CW], f32, tag="out")
                    nc.scalar.copy(ot[:], pst[:])
                    nc.gpsimd.dma_start(o_out[p], ot[:])
    nc.compile()
    return nc


_NC = None


def _get_nc():
    global _NC
    if _NC is None:
        _NC = _build_program()
    return _NC


def _pack_inputs(A, B):
    """Per-core in_maps from full A, B."""
    A = np.ascontiguousarray(np.asarray(A, dtype=np.float32))
    B = np.ascontiguousarray(np.asarray(B, dtype=np.float32))
    # AT[m, kt] = A[m-strip, kt-tile].T  -> [NM, NM, P(k), P(m)]
    AT = A.reshape(NM, P, NM, P).transpose(0, 2, 3, 1)
    # BW[j, c2] -> [P(k), WK(t), CW]
    Brs = B.reshape(NM // WK, WK, P, NC2, CW)
    in_maps = []
    for windows, pieces in _CORES:
        a_arr = np.empty((NPIECES, P, WK, P), np.float32)
        for p, (c2, j, m) in enumerate(pieces):
            # a_arr[p][k, t, mm] = AT[m, 8j+t][k, mm]
            a_arr[p] = AT[m, 8 * j : 8 * j + WK].transpose(1, 0, 2)
        b_arr = np.empty((len(SLOTS), P, WK, CW), np.float32)
        for s, (c2, j) in enumerate(windows):
            b_arr[s] = Brs[j, :, :, c2, :].transpose(1, 0, 2)
        in_maps.append(
            {
                "a": a_arr.reshape(NPIECES, P, WK * P),
                "b": b_arr.reshape(len(SLOTS), P, WK * CW),
            }
        )
    return in_maps


def _unpack_output(results):
    C = np.zeros((N, N), np.float32)
    for core, (_, pieces) in enumerate(_CORES):
        o = results[core]["o"]
        for p, (c2, j, m) in enumerate(pieces):
            C[P * m : P * (m + 1), CW * c2 : CW * (c2 + 1)] += o[p]
    return C


def _run(in_maps, trace=False):
    from concourse.bass_utils import run_bass_kernel_spmd

    return run_bass_kernel_spmd(
        _get_nc(), in_maps, core_ids=list(range(NCORES)), trace=trace
    )


def kernel(A, B):
    res = _run(_pack_inputs(A, B))
    return _unpack_output(res.results)


# revision 4
# speedup vs baseline: 1.0098x; 1.0098x over previous
"""Distributed triangular matmul C = tril(tril(A) @ tril(B)), N=4096, on 8 TRN2 cores.

Decomposition: output units (m, c2) = 128-row strip x 1024-col tile, for the
block-lower-triangle (m >= 8*c2). Each unit's k-range [8*c2, m] is split into
aligned 8-k-tile windows (1024 k each) -> "pieces". All 160 pieces are
identical in shape (8 k-steps x 2 psum banks), so an identical SPMD program
runs on all 8 cores (20 pieces each, grouped 8+8+4 under 3 resident B
windows); per-core semantics live entirely in host-packed input data.
Out-of-triangle k-steps multiply zero blocks of tril A / tril B - harmless.
Host sums the piece partials per unit (<= 4 pieces/unit).

Compute dtype: float32r (TF32-like, 1-8-11 in top 20 bits) - full PE rate at
free dim 512. DVE tensor_copy fp32->fp32r performs the rounding the BIR
verifier requires, and collapses matmul sync-waits onto one DVE semaphore
(the fp32r matmul's internal LDW has a tiny sync-wait slot budget).
"""

import sys

sys.path.insert(0, "/opt/trn_rl_repo")

import numpy as np

N = 4096
P = 128                  # partition / k-tile / m-strip size
CW = 1024                # unit column width (2 psum banks)
NM = N // P              # 32 m-strips
NC2 = N // CW            # 4 column tiles
WK = 8                   # k-tiles per full window
SLOTS = ((8, 8), (8, 8), (4, 4), (2, 4))  # (pieces, k-tiles) per window slot
NPIECES = sum(n for n, _ in SLOTS)        # 22 per core
NCORES = 8


def _schedule():
    """Global list of (c2, j, m) pieces chunked into the uniform template.

    Returns per-core: windows  [3 x (c2, j)], pieces [20 x (c2, j, m)].
    """
    chunks8 = []   # ((c2, k0, wk), ms) for the two 8-k-tile slots
    for c2 in range(NC2):
        for j in range(c2, 3):
            ms = list(range(8 * j, NM))
            for s in range(0, len(ms), 8):
                chunks8.append(((c2, 8 * j, 8), ms[s : s + 8]))
    # j=3 split into two 4-k-tile windows: k-tiles 24-27 (all m) / 28-31
    chunks4a = [((c2, 24, 4), list(range(24 + 4 * h, 28 + 4 * h)))
                for c2 in range(NC2) for h in range(2)]
    chunks4b = [((c2, 28, 4), list(range(28 + 2 * h, 30 + 2 * h)))
                for c2 in range(NC2) for h in range(2)]
    assert len(chunks8) == 16 and len(chunks4a) == 8 and len(chunks4b) == 8
    cores = []
    for i in range(NCORES):
        sel = [chunks8[2 * i], chunks8[2 * i + 1], chunks4a[i], chunks4b[i]]
        windows = [w for w, _ in sel]
        pieces = [(w[0], w[1], m, w[2]) for w, ms in sel for m in ms]
        assert len(pieces) == NPIECES
        cores.append((windows, pieces))
    return cores


_CORES = _schedule()


def _build_program():
    import concourse.mybir as mybir
    import concourse.tile as tile
    from concourse import bacc

    f32 = mybir.dt.float32
    f32r = mybir.dt.float32r

    wks = [wk for n, wk in SLOTS for _ in range(n)]
    aoff = [sum(w * P for w in wks[:p]) for p in range(NPIECES)]
    boff = [sum(SLOTS[t][1] * CW for t in range(s)) for s in range(len(SLOTS))]
    nc = bacc.Bacc(None, target_bir_lowering=False)
    a_in = nc.declare_dram_parameter("a", [P, sum(w * P for w in wks)], f32, isOutput=False)
    b_in = nc.declare_dram_parameter("b", [P, sum(w * CW for _, w in SLOTS)], f32, isOutput=False)
    o_out = nc.declare_dram_parameter("o", [NPIECES, P, CW], f32, isOutput=True)

    starts = [sum(n for n, _ in SLOTS[:i]) for i in range(len(SLOTS))]
    with tile.TileContext(nc) as tc:
        with (
            tc.tile_pool(name="braw", bufs=1) as braw_pool,
            tc.tile_pool(name="bwin", bufs=2) as bwin_pool,
            tc.tile_pool(name="araw", bufs=4) as araw_pool,
            tc.tile_pool(name="ar", bufs=3) as ar_pool,
            tc.tile_pool(name="out", bufs=3) as out_pool,
            tc.tile_pool(name="ps", bufs=4, space="PSUM") as ps_pool,
        ):
            braws, bwins, araws, ats = {}, {}, {}, {}

            # Software-pipelined emission: fp32r rounding copies (DVE) for
            # piece p+1 are queued ahead of piece p's psum evacuation
            # (ScalarE), so the DVE FIFO never stalls the PE chain. Output
            # DMAs ride the gpsimd SWDGE; A/B loads the sync HWDGE.
            def b_dma(s):
                wk = SLOTS[s][1]
                braws[s] = braw_pool.tile([P, wk, CW], f32, tag="braw", name=f"braw{s}")
                src = b_in[:, boff[s] : boff[s] + wk * CW]
                nc.sync.dma_start(
                    braws[s][:], src.rearrange("p (w c) -> p w c", w=wk)
                )
                bwins[s] = bwin_pool.tile(
                    [P, wk, CW], f32r, tag="bwin", name=f"bwin{s}"
                )

            def b_round(s, c):
                nc.vector.tensor_copy(bwins[s][:, c, :], braws[s][:, c, :])

            def a_dma(p):
                wk = wks[p]
                araws[p] = araw_pool.tile([P, wk, P], f32, tag="araw", name=f"araw{p}")
                src = a_in[:, aoff[p] : aoff[p] + wk * P]
                nc.sync.dma_start(
                    araws[p][:], src.rearrange("p (w m) -> p w m", w=wk)
                )

            def a_round(p):
                wk = wks[p]
                ats[p] = ar_pool.tile([P, wk, P], f32r, tag="ar", name=f"ar{p}")
                nc.vector.tensor_copy(ats[p][:], araws[p][:])

            b_dma(0)
            for c in range(SLOTS[0][1]):
                b_round(0, c)
            a_dma(0)
            a_round(0)
            a_dma(1)

            for s, (npc, wk) in enumerate(SLOTS):
                for q in range(npc):
                    p = starts[s] + q
                    if q == 0 and s + 1 < len(SLOTS):
                        b_dma(s + 1)
                    if s + 1 < len(SLOTS):
                        nwk = SLOTS[s + 1][1]
                        cpp = (nwk + npc - 1) // npc
                        for c in range(cpp * q, min(cpp * (q + 1), nwk)):
                            b_round(s + 1, c)
                    if p + 2 < NPIECES:
                        a_dma(p + 2)
                    if p + 1 < NPIECES:
                        a_round(p + 1)
                    pst = ps_pool.tile([P, CW], f32, tag="ps")
                    for t in range(wk):
                        for h in range(2):
                            nc.tensor.matmul(
                                pst[:, 512 * h : 512 * (h + 1)],
                                ats[p][:, t, :],
                                bwins[s][:, t, 512 * h : 512 * (h + 1)],
                                start=(t == 0),
                                stop=(t == wk - 1),
                            )
                    ot = out_pool.tile([P, CW], f32, tag="out")
                    nc.scalar.copy(ot[:], pst[:])
                    nc.gpsimd.dma_start(o_out[p], ot[:])
    nc.compile()
    return nc


_NC = None


def _get_nc():
    global _NC
    if _NC is None:
        _NC = _build_program()
    return _NC


def _pack_inputs(A, B):
    """Per-core in_maps from full A, B."""
    A = np.ascontiguousarray(np.asarray(A, dtype=np.float32))
    B = np.ascontiguousarray(np.asarray(B, dtype=np.float32))
    # AT[m, kt] = A[m-strip, kt-tile].T  -> [NM, NM, P(k), P(m)]
    AT = A.reshape(NM, P, NM, P).transpose(0, 2, 3, 1)
    BT = B.reshape(NM, P, NC2, CW)
    in_maps = []
    for windows, pieces in _CORES:
        a_parts = []
        for c2, k0, m, wk in pieces:
            a_parts.append(AT[m, k0 : k0 + wk].transpose(1, 0, 2).reshape(P, wk * P))
        b_parts = []
        for c2, k0, wk in windows:
            b_parts.append(
                BT[k0 : k0 + wk, :, c2, :].transpose(1, 0, 2).reshape(P, wk * CW)
            )
        in_maps.append(
            {
                "a": np.ascontiguousarray(np.concatenate(a_parts, axis=1)),
                "b": np.ascontiguousarray(np.concatenate(b_parts, axis=1)),
            }
        )
    return in_maps


def _unpack_output(results):
    C = np.zeros((N, N), np.float32)
    for core, (_, pieces) in enumerate(_CORES):
        o = results[core]["o"]
        for p, (c2, k0, m, wk) in enumerate(pieces):
            C[P * m : P * (m + 1), CW * c2 : CW * (c2 + 1)] += o[p]
    return C


def _run(in_maps, trace=False):
    from concourse.bass_utils import run_bass_kernel_spmd

    return run_bass_kernel_spmd(
        _get_nc(), in_maps, core_ids=list(range(NCORES)), trace=trace
    )


def kernel(A, B):
    res = _run(_pack_inputs(A, B))
    return _unpack_output(res.results)
